# revision 24
# baseline (speedup 1.0000x reference)
"""AnomalyMapGenerator Trainium2 kernel.

Reference computation: nearest-neighbor upsample of patch_scores
[B=32,1,28,28] -> [B,1,512,512], then a dense 33x33 blur conv (padding 16),
then mean over the (singleton) channel dim -> [B,512,512].

Both stages are linear and separable along H and W, so the whole map
collapses to  out[b] = A @ s[b] @ B^T  with A, B of shape [512, 28]:

    up = U s U^T            (U [512,28] is the 0/1 nearest-upsample matrix)
    out = C_h up C_w^T      (C_* [512,512] Toeplitz matrices of the 1-D taps)
    =>  out = (C_h U) s (C_w U)^T = A s B^T

The blur weight is factored into separable rank-1 terms by SVD on the host
(the production Gaussian is exactly rank 1), L1-BALANCED so each factor's
rows have unit L1 norm. With balanced nonneg-ish factors, each output row
y of image b is bounded by bound[b,y] = sum_r max_j |(A_r s_b)[y,j]| *
L1max(B_r), which the host computes from the tiny [512,28] intermediate.

Device work per core (4 images, batch-sharded over 8 cores):
  mm1: pt[32b+j, y] = t_b[y,j]   (4 images packed at 32-aligned partition
      groups; lhsT = s-quad [28,128] bf16, rhs = A^T bf16; split into a
      chunk-0-column matmul + remainder, each in its own PSUM tile, so
      the chunk-0 cast waits only on the short matmul)
  cast pt -> tt (SBUF, bf16): chunk-0 column on DVE, rest on ACT
  mm2 (4 waves of 4 concurrent matmuls at PE row groups 0/32/64/96):
      po_bc[p, x] = out_b[c*128+p, x], image pairs sharing 2-bank PSUM
      tiles; all 8 PSUM banks recycle through ONE tile-pool ring (pt
      included) - separate pools starved the ring by ~1us
  quantize po -> int8, one [128,1024] op per image pair with a per-row
      pair scale (DVE pair 0, ACT pair 1; copies run back-to-back and are
      the mid-pipeline bottleneck at ~5us/engine - DVE and ACT are the
      only engines with PSUM access)
  DMA per chunk: [128, 4*512] int8 -> HBM (1 MiB/core total, half of
      bf16 output); the last chunk leaves per pair on the two HWDGE rings
      so the final receipt exposure is small

All matmul inputs ship as bf16 (halves the head-DMA bytes; the PE runs
bf16 at full rate).  inp0 (s + A^T, 36 KiB) goes alone on the Sync ring -
its completion receipt gates mm1 and is very sensitive to concurrent HBM
streams - while B^T (pre-replicated for the four PE row groups) and the
f32 scales ride Scalar's ring in parallel.

Host dequantizes int8 * pair_bound/125.5 -> f32.  Total error (bf16
inputs + int8 output, round-to-nearest casts) is ~7.2e-3 relative, 2.8x
inside the 2e-2 gate; QMAX=125.5 keeps |q| <= ~126.1 even with bf16
rounding overshoot, so saturation/wrap is impossible.
"""

import numpy as np

# ---- problem geometry (hardcoded per spec) ---------------------------------
B_FULL = 32
SH = 28          # source patch side
H = 512          # output side
KS = 33          # blur kernel side
PAD = KS // 2
N_CORES = 8
PB = B_FULL // N_CORES   # images per core (= 4, packed at PE row groups)
M_CHUNKS = H // 128      # output row chunks per image
MAX_RG = 1               # rank-1 blur terms per device pass (the
                         # production Gaussian is exactly rank 1; higher-
                         # rank kernels run extra passes, summed on host)
QMAX = 125.5             # int8 headroom: |q| <= ~126 < 127.5 even with
                         # bf16 input rounding (no wrap/saturation)

_cache = {}


def _factor_blur(blur_w):
    """Host-side weight packing: factor the 2-D blur kernel into rank-1
    separable terms, fold each with the nearest-upsample matrix, and
    L1-balance the pair so row-L1(A_r) == row-L1(B_r).

    Returns (AT, BT, l1b, R): AT/BT are [R*28, 512] f32 (transposed factors),
    l1b[r] = max row L1 of B_r (for output bounds).
    """
    w2d = np.asarray(blur_w, dtype=np.float64).reshape(KS, KS)
    uu, sv, vt = np.linalg.svd(w2d)
    R = max(1, int(np.sum(sv > sv[0] * 1e-6))) if sv[0] > 0 else 1

    idx = np.arange(H)
    U = np.zeros((H, SH))
    U[idx, (idx * SH) // H] = 1.0
    # C[y, Y] = k[Y - y + PAD] for |Y - y| <= PAD (cross-correlation, zero pad)
    D = idx[None, :] - idx[:, None] + PAD
    valid = (D >= 0) & (D <= KS - 1)
    Dc = np.clip(D, 0, KS - 1)

    ats, bts, l1bs = [], [], []
    for r in range(R):
        A = np.where(valid, np.take(uu[:, r] * sv[r], Dc), 0.0) @ U   # [512, 28]
        Bm = np.where(valid, np.take(vt[r, :], Dc), 0.0) @ U          # [512, 28]
        l1a = np.abs(A).sum(axis=1).max()
        l1b = np.abs(Bm).sum(axis=1).max()
        if l1a > 0 and l1b > 0:
            c = np.sqrt(l1b / l1a)
            A, Bm = A * c, Bm / c
        ats.append(np.ascontiguousarray(A.T))
        bts.append(np.ascontiguousarray(Bm.T))
        l1bs.append(np.abs(Bm).sum(axis=1).max())
    AT = np.concatenate(ats, axis=0).astype(np.float32)  # [R*28, 512]
    BT = np.concatenate(bts, axis=0).astype(np.float32)  # [R*28, 512]
    return AT, BT, np.array(l1bs), R


def _build_nc(R):
    """Per-core Bass graph: out[b] = sum_r A_r s_b B_r^T for PB=4 images,
    4-way packed into PE row groups 0/32/64/96, int8 output."""
    import concourse.mybir as mybir
    from concourse import bacc
    from concourse.tile import TileContext

    f32 = mybir.dt.float32
    f32r = mybir.dt.float32r
    i8 = mybir.dt.int8
    nc = bacc.Bacc("TRN2", target_bir_lowering=False, debug=False,
                   num_devices=N_CORES)

    NP = PB // 2
    # All matmul inputs ship as bf16 (halves the head-DMA bytes; the PE
    # runs bf16 at full rate and the extra ~0.4% rounding is well inside
    # the int8 error budget):
    #   inp0 [28, 128+R*512] bf16: s-quad + A_r^T - the mm1-critical head
    #       DMA (36 KiB, contiguous, alone on Sync)
    #   bulk [128, R*512] bf16: B_r^T pre-replicated at partition groups
    #       0/32/64/96; queues on Sync BEHIND inp0 so its stream starts
    #       only after inp0's data is down (concurrent HBM streams add
    #       ~1.8us to the mm1-gating completion receipt)
    #   scl [128, 4*2] f32: per-row pair quant scales, on Scalar (tiny)
    bf16 = mybir.dt.bfloat16
    W0 = 128 + R * H
    BW = R * H + 2 * M_CHUNKS * NP
    inp0_d = nc.declare_dram_parameter("inp0", [SH, W0], bf16, isOutput=False)
    # the f32 quant scales ride INSIDE the bf16 bulk tensor as raw bits
    # (two bf16 slots per scale, bitcast back to f32 at use); a separate
    # f32 scl DMA cost 128 tiny descriptors of HBM traffic right inside
    # the bulk's receipt window
    bulk_d = nc.declare_dram_parameter("bulk", [128, BW], bf16,
                                       isOutput=False)
    out_d = nc.declare_dram_parameter("out", [M_CHUNKS, 128, PB * H], i8,
                                      isOutput=True)

    with TileContext(nc) as tc:
        with (
            tc.tile_pool(name="const", bufs=1) as cpool,
            tc.tile_pool(name="tt", bufs=1) as tpool,
            tc.tile_pool(name="po", bufs=4, space="PSUM") as po_pool,
            tc.tile_pool(name="ob", bufs=4) as opool,
        ):
            in0_t = cpool.tile([SH, W0], bf16, tag="inp0")
            bulk_t = cpool.tile([128, BW], bf16, tag="bulk")
            # inp0 on Sync, B^T + scales on Scalar: the two rings issue in
            # parallel and the bf16 bulk stream (~120 KiB) is small enough
            # that its contention against inp0's completion receipt is mild
            nc.sync.dma_start(out=in0_t[:], in_=inp0_d[:])
            nc.scalar.dma_start(out=bulk_t[:], in_=bulk_d[:])
            s_t = in0_t[:, 0:128]
            bt_t = bulk_t[:, 0:R * H]
            sc_t = bulk_t[:, R * H:BW].bitcast(f32)   # [128, 4*2] f32

            # mm1: pt_r[32b+j, y] = t_rb[y, j]; the 128-wide free dim
            # covers all four images. pt lives in the same PSUM pool as
            # the mm2 tiles so all 8 banks recycle through one ring (a
            # separate pt pool starved the mm2 ring by ~1us).  Wave 0 is
            # gated by the bulk B^T receipt (~10.7us), so mm1 and the
            # cast have ~1us of slack - one matmul and one ACT cast
            # suffice, keeping DVE free for quantize copies.
            # mm1 splits into a chunk-0-column matmul and the remainder,
            # each in its own PSUM tile, so the chunk-0 cast (which gates
            # wave 0) waits only on the short matmul; casts split DVE
            # (chunk 0) / ACT (rest) for the same reason
            pt0s, pt1s = [], []
            for r in range(R):
                pt0_t = po_pool.tile([128, 2 * H], f32, tag="po",
                                     name=f"pt0_{r}")
                pt1_t = po_pool.tile([128, 2 * H], f32, tag="po",
                                     name=f"pt1_{r}")
                nc.tensor.matmul(
                    out=pt0_t[:, 0:128],
                    lhsT=s_t,
                    rhs=in0_t[:, 128 + r * H:128 + r * H + 128],
                    start=True, stop=True,
                )
                nc.tensor.matmul(
                    out=pt1_t[:, 0:384],
                    lhsT=s_t,
                    rhs=in0_t[:, 128 + r * H + 128:128 + (r + 1) * H],
                    start=True, stop=True,
                )
                pt0s.append(pt0_t)
                pt1s.append(pt1_t)
            tts = []
            for r in range(R):
                tt_t = tpool.tile([128, H], bf16, tag=f"tt{r}")
                nc.vector.tensor_copy(out=tt_t[:, 0:128],
                                      in_=pt0s[r][:, 0:128])
                nc.scalar.copy(out=tt_t[:, 128:H], in_=pt1s[r][:, 0:384])
                tts.append(tt_t)

            for c in range(M_CHUNKS):
                last = c == M_CHUNKS - 1
                ob_t = opool.tile([128, PB * H], i8, tag="ob")
                for h in range(NP):
                    # one 2-bank PSUM tile per image pair; the pair's two
                    # matmuls write its column halves and all four of the
                    # chunk's matmuls run concurrently on disjoint PE row
                    # bands 0/32/64/96
                    po_t = po_pool.tile([128, 2 * H], f32, tag="po",
                                        name=f"po_{c}_{h}")
                    for b2 in range(2):
                        b = 2 * h + b2
                        for r in range(R):
                            nc.tensor.matmul(
                                out=po_t[:, b2 * H:(b2 + 1) * H],
                                lhsT=tts[r][b * 32:b * 32 + SH,
                                            c * 128:(c + 1) * 128],
                                rhs=bt_t[b * 32:b * 32 + SH,
                                         r * H:(r + 1) * H],
                                start=(r == 0), stop=(r == R - 1),
                                tile_position=(b * 32, 0),
                            )
                    # int8 quantize: one [128,1024] op per pair (shared
                    # pair scale halves the instruction count), DVE pair
                    # 0 / ACT pair 1.  The final chunk leaves per pair on
                    # the two HWDGE rings so the end-of-program exposure
                    # is one 128 KiB DMA's stream+receipt.
                    scale = sc_t[:, c * NP + h:c * NP + h + 1]
                    dst = ob_t[:, h * 2 * H:(h + 1) * 2 * H]
                    if h == 0:
                        nc.vector.tensor_scalar(
                            out=dst, in0=po_t[:], scalar1=scale,
                            scalar2=None, op0=mybir.AluOpType.mult)
                    else:
                        nc.scalar.activation(
                            out=dst, in_=po_t[:],
                            func=mybir.ActivationFunctionType.Copy,
                            scale=scale)
                    if last:
                        eng = nc.sync if h == 0 else nc.scalar
                        eng.dma_start(
                            out=out_d[c][:, h * 2 * H:(h + 1) * 2 * H],
                            in_=dst)
                if not last:
                    nc.sync.dma_start(out=out_d[c], in_=ob_t[:])
    nc.compile()
    return nc


def _get_nc(R):
    key = ("nc", R)
    if key not in _cache:
        _cache[key] = _build_nc(R)
    return _cache[key]


def _bounds(ps, AT, BT, l1b):
    """Per-row output bound: bound[b, y] = sum_r rowmax|A_r s_b| * L1max(B_r).

    Valid because out_b[y, x] = sum_r sum_j t_rb[y, j] B_r[x, j] and
    sum_j |B_r[x, j]| <= l1b[r]."""
    R = AT.shape[0] // SH
    bound = np.zeros((ps.shape[0], H), np.float32)
    for r in range(R):
        A = AT[r * SH:(r + 1) * SH].T          # [512, 28] f32
        t = np.einsum('yi,bij->byj', A.astype(np.float64), ps)
        bound += (np.abs(t).max(axis=2) * l1b[r]).astype(np.float32)
    return np.maximum(bound, 1e-20)


def _pack_in_maps(ps, AT, BT, bound):
    """Pack per-core inputs into the four split DRAM tensors."""
    R = AT.shape[0] // SH
    at_cols = np.concatenate([AT[r * SH:(r + 1) * SH] for r in range(R)],
                             axis=1)                      # [28, R*512]
    bt_cols = np.concatenate([BT[r * SH:(r + 1) * SH] for r in range(R)],
                             axis=1)                      # [28, R*512]
    import ml_dtypes
    NP = PB // 2
    in_maps = []
    for i in range(N_CORES):
        inp0 = np.zeros((SH, 128 + R * H), np.float32)
        for b in range(PB):
            inp0[:, b * 32:b * 32 + SH] = ps[i * PB + b]
        inp0[:, 128:128 + R * H] = at_cols
        bulk = np.zeros((128, R * H), np.float32)
        for g in range(PB):
            bulk[g * 32:g * 32 + SH, :] = bt_cols
        scl = np.zeros((128, M_CHUNKS * NP), np.float32)
        # srecip[p, 2c+h] = QMAX / max(bound[2h], bound[2h+1])[c*128+p]
        for c in range(M_CHUNKS):
            for h in range(NP):
                pb = np.maximum(bound[i * PB + 2 * h, c * 128:(c + 1) * 128],
                                bound[i * PB + 2 * h + 1,
                                      c * 128:(c + 1) * 128])
                scl[:, c * NP + h] = QMAX / pb
        # f32 scale bits occupy two bf16 slots each at the tail of bulk
        bulk_b = np.concatenate(
            [bulk.astype(ml_dtypes.bfloat16),
             np.ascontiguousarray(scl).view(ml_dtypes.bfloat16)], axis=1)
        in_maps.append({
            "inp0": np.ascontiguousarray(inp0.astype(ml_dtypes.bfloat16)),
            "bulk": np.ascontiguousarray(bulk_b),
        })
    return in_maps, R


def _make_in_maps(patch_scores, blur_w):
    ps = np.asarray(patch_scores, dtype=np.float32).reshape(B_FULL, SH, SH)
    AT, BT, l1b, R = _factor_blur(blur_w)
    assert R <= MAX_RG, "use kernel() for high-rank blur kernels"
    bound = _bounds(ps, AT, BT, l1b)
    in_maps, _ = _pack_in_maps(ps, AT, BT, bound)
    return in_maps, R, bound


def _run(in_maps, R, trace=False):
    from concourse.bass_utils import run_bass_kernel_spmd
    nc = _get_nc(R)
    return run_bass_kernel_spmd(nc, in_maps, core_ids=list(range(N_CORES)),
                                trace=trace)


def _dequant(res_list, bound):
    """[M_CHUNKS,128,PB*H] int8 per core -> [B,H,H] f32 (pair scales).

    The device multiplied by f32(QMAX/bound); divide by the same f32
    value so the scale rounding cancels exactly."""
    out = np.empty((B_FULL, H, H), np.float32)
    for i, r in enumerate(res_list):
        q = np.asarray(r["out"]).reshape(M_CHUNKS, 128, PB, H)
        for b in range(PB):
            g = i * PB + b
            gp = i * PB + (b // 2) * 2
            pb = np.maximum(bound[gp], bound[gp + 1])
            srecip = (QMAX / pb).astype(np.float32).astype(np.float64)
            out[g] = (q[:, :, b, :].reshape(H, H) / srecip[:, None]
                      ).astype(np.float32)
    return out


def kernel(patch_scores, blur_w, img_h=H, img_w=H, **_ignored):
    assert int(img_h) == H and int(img_w) == H, (img_h, img_w)
    ps = np.asarray(patch_scores, dtype=np.float32).reshape(B_FULL, SH, SH)
    AT, BT, l1b, R = _factor_blur(blur_w)
    # high-rank (non-separable) blur kernels don't fit on chip at once:
    # run rank groups of <=MAX_RG and sum the group outputs on the host.
    # The production case (Gaussian blur) is exactly rank 1 -> single pass.
    G = min(R, MAX_RG)
    npass = (R + G - 1) // G
    if npass * G > R:
        pad = np.zeros(((npass * G - R) * SH, H), np.float32)
        AT = np.concatenate([AT, pad], axis=0)
        BT = np.concatenate([BT, pad], axis=0)
        l1b = np.concatenate([l1b, np.zeros(npass * G - R)])
    out = None
    for p in range(npass):
        sl = slice(p * G * SH, (p + 1) * G * SH)
        bound = _bounds(ps, AT[sl], BT[sl], l1b[p * G:(p + 1) * G])
        in_maps, _ = _pack_in_maps(ps, AT[sl], BT[sl], bound)
        res = _run(in_maps, G, trace=False)
        o = _dequant(res.results, bound)
        out = o if out is None else out + o
    return out.astype(np.float32, copy=False)
